# revision 20
# baseline (speedup 1.0000x reference)
"""
Trainium2 Bass kernel for nn_Encoder (embedding lookup + LSTM, returns final (h, c)).

Strategy (data-parallel over batch, per sharding hint): 8 cores, each
handling B_local = 4 of the 32 batch rows. Active version: v4
(build_program_v4, KERNEL_VER below); v2/v3 builders kept for reference.

v4 per core:
  - Prep (overlapped with the recurrence, one item per step): gather
    embedding rows via indirect DMA, transpose on PE, project x @ W in
    fp16 (N=512 moving so the ~32ns fixed per-matmul cost amortizes),
    scatter xz + bias into fp16 in the per-step packed layout.
  - Recurrence (measured PE-bound: each InstMatmult costs ~32ns fixed on
    HW regardless of stationary reuse, K, or accumulation grouping, so
    the 65 matmuls/step dominate): per step two psum tiles [128, 32]
    (hs-pairs), xz injected via identity-matmul (start=True) then 64
    U-matmuls accumulate z^T; g-gate columns of W/U/b are pre-doubled
    host-side so ONE sigmoid per pair covers i,f,o and sg=(tanh(z_g)+1)/2;
    cell state kept as cc=2c and h stored as h/2 in fp16 (U pre-doubled
    compensates) which turns the whole cell update into 4 fused DVE ops +
    1 sigmoid per pair:
      t1 = f*cc; t2 = (sg-0.5)*i; cc = 4*t2 + t1; v = sigmoid(cc);
      h/2 = (v-0.5)*o
  - h16 is split into two tiles (hs-pairs) and matmuls are ordered
    hs01-first within each k so the next step's PE work starts as soon
    as the first pair's h lands.

Host side: shard/marshal + pre-scale inputs, run SPMD on 8 cores,
unpack and rescale outputs (h*2, c/2).
"""

import numpy as np

import concourse.bass as bass
import concourse.mybir as mybir
import concourse.tile as tile
from concourse import bacc
from concourse.bass import IndirectOffsetOnAxis
from concourse.bass_utils import run_bass_kernel_spmd
from concourse.masks import make_identity

# Problem constants (hardcoded; harness contract)
B, T, V, E, H = 32, 512, 20000, 300, 512
G4 = 4 * H            # 2048
NCORES = 8
BL = B // NCORES      # 4 batch rows per core
P = 128
KM = G4 // P          # 16 M-tiles over 4H
KH = H // P           # 4 K-tiles over H
KE_SIZES = [128, 128, 44]   # K subtiles over E=300
# Keras gate g (i,f,g,o) -> packed slot (i,f,o,g): sigmoid = slots 0..2, tanh = slot 3
PERM = [0, 1, 3, 2]

f32 = mybir.dt.float32
f32r = mybir.dt.float32r
f16 = mybir.dt.float16
i32 = mybir.dt.int32

AF = mybir.ActivationFunctionType


def build_program(nc, T_steps=T, Tc=128, dbg_step=None, reps=1, sched="v2",
                  loop_reps=0):
    """Emit the full per-core program into nc (a bacc.Bacc).

    reps > 1 repeats the whole compute (for timing amplification)."""
    assert T_steps % Tc == 0
    nch = T_steps // Tc
    NJ = Tc * BL // P  # gathers (128-row groups) per chunk

    emb_t = nc.declare_dram_parameter("emb", [V, E], f32, isOutput=False)
    W_t = nc.declare_dram_parameter("W", [E, G4], f32, isOutput=False)
    U_t = nc.declare_dram_parameter("U", [H, G4], f32, isOutput=False)
    b_t = nc.declare_dram_parameter("bvec", [G4], f32, isOutput=False)
    tok_t = nc.declare_dram_parameter("tok", [P, T_steps * BL // P], i32, isOutput=False)
    ho_t = nc.declare_dram_parameter("ho", [P, BL * KH], f16, isOutput=True)
    co_t = nc.declare_dram_parameter("co", [P, BL * KH], f32, isOutput=True)
    if dbg_step is not None:
        dbg_z = nc.declare_dram_parameter("dbg_z", [P, 64], f32, isOutput=True)
        dbg_h = nc.declare_dram_parameter("dbg_h", [P, BL * KH], f16, isOutput=True)
        dbg_c = nc.declare_dram_parameter("dbg_c", [P, BL * KH], f32, isOutput=True)

    with tile.TileContext(nc) as tc:
        with (
            tc.tile_pool(name="const", bufs=1) as cpool,
            tc.tile_pool(name="ustage", bufs=2) as upool,
            tc.tile_pool(name="xrows", bufs=4) as xpool,
            tc.tile_pool(name="xtp", bufs=2) as xtpool,
            tc.tile_pool(name="ptr", bufs=2, space="PSUM") as ptr_pool,
            tc.tile_pool(name="pxz", bufs=2, space="PSUM") as pxz_pool,
            tc.tile_pool(name="pz", bufs=4, space="PSUM") as pz_pool,
        ):
            # ---- constants / weights ----
            U16 = cpool.tile([P, KH * G4], f16, tag="U16")
            W_sb = cpool.tile([P, 3 * G4], f16, tag="Wsb")
            b_sb = cpool.tile([P, KM], f32, tag="bsb")
            tok_sb = cpool.tile([P, T_steps * BL // P], i32, tag="tok")
            ident = cpool.tile([P, P], f32, tag="ident")
            h16 = cpool.tile([P, BL * KH], f16, tag="h16")
            cst = cpool.tile([P, BL * KH], f32, tag="cst")
            z_s = cpool.tile([P, 64], f32, tag="zs")
            a_s = cpool.tile([P, 64], f32, tag="as")
            tmp1 = cpool.tile([P, BL * KH], f32, tag="t1")
            tmp2 = cpool.tile([P, BL * KH], f32, tag="t2")
            tct = cpool.tile([P, BL * KH], f32, tag="tct")
            xz_sb = [
                cpool.tile([P, Tc * 64], f32, tag=f"xz{par}", name=f"xz{par}")
                for par in range(2)
            ]

            make_identity(nc, ident[:])

            # U (fp32 DRAM) -> U16 (fp16 SBUF), K-tile k region at cols k*G4
            for k in range(KH):
                ust = upool.tile([P, G4], f32, tag="ustage")
                nc.sync.dma_start(ust[:], U_t.ap()[k * P:(k + 1) * P, :])
                nc.vector.tensor_copy(U16[:, k * G4:(k + 1) * G4], ust[:])

            # W: 3 K-subtiles at cols kk*G4, cast to fp16 via staging
            ofs = 0
            for kk, kw in enumerate(KE_SIZES):
                wst = upool.tile([P, G4], f32, tag="ustage", name=f"wst{kk}")
                nc.sync.dma_start(wst[:kw, :], W_t.ap()[ofs:ofs + kw, :])
                nc.vector.tensor_copy(W_sb[:kw, kk * G4:(kk + 1) * G4], wst[:kw, :])
                ofs += kw

            # bias: b_sb[p, m] = b[m*128 + p]
            nc.sync.dma_start(b_sb[:], b_t.ap().rearrange("(m p) -> p m", p=P))
            nc.sync.dma_start(tok_sb[:], tok_t.ap())

            nc.gpsimd.memset(h16[:], 0.0)
            nc.gpsimd.memset(cst[:], 0.0)

            def emit_prep(c):
                """Gather + transpose + xz projection for chunk c."""
                xz_dst = xz_sb[c % 2]
                xT = xtpool.tile([P, 3 * Tc * BL], f16, tag="xT")
                for j in range(NJ):
                    xr = xpool.tile([P, E], f32, tag="xrows")
                    nc.gpsimd.indirect_dma_start(
                        out=xr[:],
                        out_offset=None,
                        in_=emb_t.ap(),
                        in_offset=IndirectOffsetOnAxis(
                            ap=tok_sb[:, c * NJ + j:c * NJ + j + 1], axis=0
                        ),
                    )
                    for kk, kw in enumerate(KE_SIZES):
                        pt = ptr_pool.tile([P, P], f32, tag="ptr")
                        nc.tensor.transpose(
                            out=pt[:kw, :], in_=xr[:, kk * P:kk * P + kw],
                            identity=ident[:],
                        )
                        nc.vector.tensor_copy(
                            xT[:kw, kk * Tc * BL + j * P:kk * Tc * BL + (j + 1) * P],
                            pt[:kw, :],
                        )
                N = Tc * BL
                for m in range(KM):
                    pxz = pxz_pool.tile([P, N], f32, tag="pxz")
                    for kk, kw in enumerate(KE_SIZES):
                        nc.tensor.matmul(
                            pxz[:],
                            W_sb[:kw, kk * G4 + m * P:kk * G4 + (m + 1) * P],
                            xT[:kw, kk * N:(kk + 1) * N],
                            start=(kk == 0),
                            stop=(kk == 2),
                        )
                    # packed dest: col = t*64 + (m%4)*16 + PERM[m//4]*4 + b
                    slot = (m % 4) * 16 + PERM[m // 4] * 4
                    dst = xz_dst[:].rearrange("p (t g) -> p t g", g=64)[
                        :, :, slot:slot + 4
                    ]
                    src = pxz[:].rearrange("p (t b) -> p t b", b=BL)
                    nc.vector.tensor_scalar_add(dst, src, b_sb[:, m:m + 1])

            # MM emission order for the last K round: group M-tiles by H-slice
            ORDER_LAST = [m for hs in range(4) for m in (hs, 4 + hs, 8 + hs, 12 + hs)]

            def emit_step_v1(c, t):
                psz = [
                    pz_pool.tile([P, 16], f32, tag="pz", name=f"pz{hs}_{c}_{t}")
                    for hs in range(4)
                ]
                for k in range(KH):
                    order = ORDER_LAST if k == KH - 1 else range(KM)
                    for m in order:
                        slot = PERM[m // 4] * 4
                        # start=True marks the whole 2KB psum bank pending-zero,
                        # so only the FIRST matmul touching each psz tile sets it
                        # (round k=0, m in 0..3); later slots overwrite via
                        # pending-zero, later k rounds accumulate.
                        nc.tensor.matmul(
                            psz[m % 4][:, slot:slot + 4],
                            U16[:, k * G4 + m * P:k * G4 + (m + 1) * P],
                            h16[:, k * BL:(k + 1) * BL],
                            start=(k == 0 and m < 4),
                            stop=(k == KH - 1),
                            skip_group_check=True,
                        )
                for hs in range(4):
                    zs = z_s[:, hs * 16:hs * 16 + 16]
                    nc.vector.tensor_add(
                        zs,
                        psz[hs][:],
                        xz_sb[c % 2][:, t * 64 + hs * 16:t * 64 + hs * 16 + 16],
                    )
                    # sigmoid over (i, f, o) slots, tanh over g slot
                    nc.scalar.activation(
                        a_s[:, hs * 16:hs * 16 + 12], z_s[:, hs * 16:hs * 16 + 12],
                        AF.Sigmoid,
                    )
                    nc.scalar.activation(
                        a_s[:, hs * 16 + 12:hs * 16 + 16],
                        z_s[:, hs * 16 + 12:hs * 16 + 16],
                        AF.Tanh,
                    )
                    cs = slice(hs * BL, (hs + 1) * BL)
                    nc.vector.tensor_mul(
                        tmp1[:, cs], a_s[:, hs * 16 + 4:hs * 16 + 8], cst[:, cs]
                    )  # f * c
                    nc.vector.tensor_mul(
                        tmp2[:, cs],
                        a_s[:, hs * 16:hs * 16 + 4],
                        a_s[:, hs * 16 + 12:hs * 16 + 16],
                    )  # i * g
                    nc.vector.tensor_add(cst[:, cs], tmp1[:, cs], tmp2[:, cs])
                    nc.scalar.activation(tct[:, cs], cst[:, cs], AF.Tanh)
                    nc.vector.tensor_mul(
                        h16[:, cs], a_s[:, hs * 16 + 8:hs * 16 + 12], tct[:, cs]
                    )  # h = o * tanh(c), cast to fp16 on write

            def a2(base, width):
                """2D AP over a_s/z_s: [128, (2 hs, width)] at col base within
                each 16-col hs block of the pair being processed."""
                return base.rearrange("p (hs w) -> p hs w", w=16)

            def emit_step_v2(c, t):
                # 2 psum tiles, one per hs-pair; cols = (hs%2)*16 + slot*4 + b
                psz = [
                    pz_pool.tile([P, 32], f32, tag="pz", name=f"pzp{pr}_{c}_{t}")
                    for pr in range(2)
                ]
                # pair-major PE order: all of pair 0's MMs (k-outer), then pair 1
                for pr in range(2):
                    for k in range(KH):
                        for hs in (2 * pr, 2 * pr + 1):
                            for g in range(4):
                                m = g * 4 + hs
                                slot = (hs % 2) * 16 + PERM[g] * 4
                                nc.tensor.matmul(
                                    psz[pr][:, slot:slot + 4],
                                    U16[:, k * G4 + m * P:k * G4 + (m + 1) * P],
                                    h16[:, k * BL:(k + 1) * BL],
                                    start=(k == 0 and hs == 2 * pr and g == 0),
                                    stop=(k == KH - 1),
                                    skip_group_check=True,
                                )
                xz = xz_sb[c % 2]
                for pr in range(2):
                    # per-hs adds (start as soon as that hs' slots are done)
                    for hs in (2 * pr, 2 * pr + 1):
                        nc.vector.tensor_add(
                            z_s[:, hs * 16:hs * 16 + 16],
                            psz[pr][:, (hs % 2) * 16:(hs % 2) * 16 + 16],
                            xz[:, t * 64 + hs * 16:t * 64 + hs * 16 + 16],
                        )
                    h0 = 2 * pr * 16  # base col of this pair in z_s/a_s
                    zs2 = z_s[:].rearrange("p (hs w) -> p hs w", w=16)
                    as2 = a_s[:].rearrange("p (hs w) -> p hs w", w=16)
                    # sigmoid over (i,f,o) of both hs in one 2D-AP instr
                    nc.scalar.activation(
                        as2[:, 2 * pr:2 * pr + 2, 0:12],
                        zs2[:, 2 * pr:2 * pr + 2, 0:12],
                        AF.Sigmoid,
                    )
                    nc.scalar.activation(
                        as2[:, 2 * pr:2 * pr + 2, 12:16],
                        zs2[:, 2 * pr:2 * pr + 2, 12:16],
                        AF.Tanh,
                    )
                    cs = slice(pr * 2 * BL, (pr + 1) * 2 * BL)  # 8 cols of cst
                    c2 = cst[:, cs].rearrange("p (hs b) -> p hs b", b=BL)
                    t1 = tmp1[:, cs].rearrange("p (hs b) -> p hs b", b=BL)
                    t2 = tmp2[:, cs].rearrange("p (hs b) -> p hs b", b=BL)
                    nc.vector.tensor_mul(
                        t1, as2[:, 2 * pr:2 * pr + 2, 4:8], c2
                    )  # f * c
                    nc.vector.tensor_mul(
                        t2,
                        as2[:, 2 * pr:2 * pr + 2, 0:4],
                        as2[:, 2 * pr:2 * pr + 2, 12:16],
                    )  # i * g
                    nc.vector.tensor_add(cst[:, cs], tmp1[:, cs], tmp2[:, cs])
                    nc.scalar.activation(tct[:, cs], cst[:, cs], AF.Tanh)
                    nc.vector.tensor_mul(
                        h16[:, cs].rearrange("p (hs b) -> p hs b", b=BL),
                        as2[:, 2 * pr:2 * pr + 2, 8:12],
                        tct[:, cs].rearrange("p (hs b) -> p hs b", b=BL),
                    )  # h = o * tanh(c), cast to fp16 on write

            emit_step = emit_step_v1 if sched == "v1" else emit_step_v2

            def v2_body(rep=0, force_memset=False):
                if rep > 0 or force_memset:
                    nc.gpsimd.memset(h16[:], 0.0)
                    nc.gpsimd.memset(cst[:], 0.0)
                emit_prep(0)
                for c in range(nch):
                    for t in range(Tc):
                        emit_step(c, t)
                        if dbg_step is not None and (c, t) == dbg_step:
                            nc.sync.dma_start(dbg_z.ap(), z_s[:])
                            nc.sync.dma_start(dbg_h.ap(), h16[:])
                            nc.sync.dma_start(dbg_c.ap(), cst[:])
                        if t == 16 and c + 1 < nch:
                            emit_prep(c + 1)

            if loop_reps > 0:
                with tc.For_i(0, loop_reps):
                    v2_body(force_memset=True)
            else:
                for rep in range(reps):
                    v2_body(rep)

            nc.sync.dma_start(ho_t.ap(), h16[:])
            nc.sync.dma_start(co_t.ap(), cst[:])

    return nc


def build_program_v3(nc, T_steps=T, Tc=128, reps=1, loop_reps=0, dbg_step=None,
                     prep_on_gpsimd=False, groups=2):
    """v3: two staggered batch-groups (A=b0,b1 / B=b2,b3) with short
    per-step chains.

    Per group-step: PE injects xz via identity-matmul (start=True) then 64
    U-matmuls accumulate z^T in one psum tile [128, 32] (cols s*8+hs*2+b2,
    s in (i,f,o,sg)); one ACT sigmoid covers all 32 cols (g-columns of
    W/U/b are pre-doubled host-side so sg = sigmoid(2 z_g) and
    g = 2*sg - 1); DVE updates cc = 2c via
      t1 = f (*) cc;  t2 = (sg - 0.5) (*) i;  cc = 4*t2 + t1
    then ACT computes v = sigmoid(cc) (= (tanh(c)+1)/2) and DVE writes
    h/2 = (v - 0.5) (*) o to fp16 (U pre-scaled 2x compensates; host
    multiplies the h output by 2 and halves the c output).

    The A and B chains interleave so engine latencies of one group hide
    under the other group's work.
    """
    assert T_steps % Tc == 0
    nch = T_steps // Tc
    NJ = Tc * BL // P
    BG = BL // groups  # batches per group
    GL = "AB"[:groups]
    ZW = 16 * BG  # z cols per group

    emb_t = nc.declare_dram_parameter("emb", [V, E], f32, isOutput=False)
    W_t = nc.declare_dram_parameter("W", [E, G4], f32, isOutput=False)
    U_t = nc.declare_dram_parameter("U", [H, G4], f32, isOutput=False)
    b_t = nc.declare_dram_parameter("bvec", [G4], f32, isOutput=False)
    tok_t = nc.declare_dram_parameter("tok", [P, T_steps * BL // P], i32,
                                      isOutput=False)
    ho_t = nc.declare_dram_parameter("ho", [P, 16], f16, isOutput=True)
    co_t = nc.declare_dram_parameter("co", [P, 16], f32, isOutput=True)
    if dbg_step is not None:
        dbg_a = nc.declare_dram_parameter("dbg_a", [P, 64], f32, isOutput=True)
        dbg_c = nc.declare_dram_parameter("dbg_c", [P, 16], f32, isOutput=True)
        dbg_h = nc.declare_dram_parameter("dbg_h", [P, 16], f16, isOutput=True)

    with tile.TileContext(nc) as tc:
        with (
            tc.tile_pool(name="const", bufs=1) as cpool,
            tc.tile_pool(name="ustage", bufs=2) as upool,
            tc.tile_pool(name="xrows", bufs=4) as xpool,
            tc.tile_pool(name="xtp", bufs=2) as xtpool,
            tc.tile_pool(name="av", bufs=3) as apool,
            tc.tile_pool(name="ptr", bufs=2, space="PSUM") as ptr_pool,
            tc.tile_pool(name="pxz", bufs=2, space="PSUM") as pxz_pool,
            tc.tile_pool(name="pz", bufs=4, space="PSUM") as pz_pool,
        ):
            U16 = cpool.tile([P, KH * G4], f16, tag="U16")
            W_sb = cpool.tile([P, 3 * G4], f16, tag="Wsb")
            b_sb = cpool.tile([P, KM], f32, tag="bsb")
            tok_sb = cpool.tile([P, T_steps * BL // P], i32, tag="tok")
            ident = cpool.tile([P, P], f32, tag="ident")
            ident16 = cpool.tile([P, P], f16, tag="ident16")
            h16 = {g: cpool.tile([P, KH * BG], f16, tag=f"h16{g}",
                                 name=f"h16{g}") for g in GL}
            cc = {g: cpool.tile([P, 4 * BG], f32, tag=f"cc{g}",
                                name=f"cc{g}") for g in GL}
            tm1 = {g: cpool.tile([P, 4 * BG], f32, tag=f"tm1{g}",
                                 name=f"tm1{g}") for g in GL}
            tm2 = {g: cpool.tile([P, 4 * BG], f32, tag=f"tm2{g}",
                                 name=f"tm2{g}") for g in GL}
            xz_sb = [cpool.tile([P, Tc * 64], f16, tag=f"xz{par}",
                                name=f"xz{par}") for par in range(2)]
            assert groups * ZW == 64

            make_identity(nc, ident[:])
            nc.vector.tensor_copy(ident16[:], ident[:])

            for k in range(KH):
                ust = upool.tile([P, G4], f32, tag="ustage")
                nc.sync.dma_start(ust[:], U_t.ap()[k * P:(k + 1) * P, :])
                nc.vector.tensor_copy(U16[:, k * G4:(k + 1) * G4], ust[:])
            ofs = 0
            for kk, kw in enumerate(KE_SIZES):
                wst = upool.tile([P, G4], f32, tag="ustage", name=f"wst{kk}")
                nc.sync.dma_start(wst[:kw, :], W_t.ap()[ofs:ofs + kw, :])
                nc.vector.tensor_copy(W_sb[:kw, kk * G4:(kk + 1) * G4],
                                      wst[:kw, :])
                ofs += kw
            nc.sync.dma_start(b_sb[:],
                              b_t.ap().rearrange("(m p) -> p m", p=P))
            nc.sync.dma_start(tok_sb[:], tok_t.ap())

            def emit_prep_items(c):
                """Yield closures: gather+transpose+project chunk c into
                xz_sb[c % 2] (layout col = t*64 + grp*32 + s*8 + hs*2 + b2)."""
                xz_dst = xz_sb[c % 2]
                xT = xtpool.tile([P, 3 * Tc * BL], f16, tag="xT")

                def gather_j(j):
                    xr = xpool.tile([P, E], f32, tag="xrows")
                    nc.gpsimd.indirect_dma_start(
                        out=xr[:], out_offset=None, in_=emb_t.ap(),
                        in_offset=IndirectOffsetOnAxis(
                            ap=tok_sb[:, c * NJ + j:c * NJ + j + 1], axis=0),
                    )
                    return xr

                xrs = {}
                for j in range(NJ):
                    def g(j=j):
                        xrs[j] = gather_j(j)
                    yield g
                for j in range(NJ):
                    for kk, kw in enumerate(KE_SIZES):
                        def tr(j=j, kk=kk, kw=kw):
                            pt = ptr_pool.tile([P, P], f32, tag="ptr")
                            nc.tensor.transpose(
                                out=pt[:kw, :], in_=xrs[j][:, kk * P:kk * P + kw],
                                identity=ident[:])
                            nc.vector.tensor_copy(
                                xT[:kw, kk * Tc * BL + j * P:
                                   kk * Tc * BL + (j + 1) * P], pt[:kw, :])
                        yield tr
                N = Tc * BL
                for m in range(KM):
                    def mm_m(m=m):
                        pxz = pxz_pool.tile([P, N], f32, tag="pxz")
                        for kk, kw in enumerate(KE_SIZES):
                            nc.tensor.matmul(
                                pxz[:],
                                W_sb[:kw, kk * G4 + m * P:kk * G4 + (m + 1) * P],
                                xT[:kw, kk * N:(kk + 1) * N],
                                start=(kk == 0), stop=(kk == 2),
                            )
                        slot = PERM[m // 4] * 4 * BG + (m % 4) * BG
                        dst = xz_dst[:].rearrange(
                            "p (t g q) -> p t g q", g=groups, q=ZW
                        )[:, :, :, slot:slot + BG]
                        src = pxz[:].rearrange("p (t bh bl) -> p t bh bl",
                                               bh=groups, bl=BG)
                        eng = nc.gpsimd if prep_on_gpsimd else nc.vector
                        eng.tensor_scalar_add(dst, src, b_sb[:, m:m + 1])
                    yield mm_m

            def emit_group_pe(g, t, c):
                pz = pz_pool.tile([P, ZW], f32, tag="pz", name=f"pz{g}_{c}_{t}")
                goff = (0 if g == "A" else 1) * ZW
                nc.tensor.matmul(
                    pz[:], ident16[:],
                    xz_sb[c % 2][:, t * 64 + goff:t * 64 + goff + ZW],
                    start=True, stop=False, skip_group_check=True,
                )
                for k in range(KH):
                    for m in range(KM):
                        slot = PERM[m // 4] * 4 * BG + (m % 4) * BG
                        nc.tensor.matmul(
                            pz[:, slot:slot + BG],
                            U16[:, k * G4 + m * P:k * G4 + (m + 1) * P],
                            h16[g][:, k * BG:(k + 1) * BG],
                            start=False, stop=(k == KH - 1),
                            skip_group_check=True,
                        )
                return pz

            Amul = mybir.AluOpType.mult
            Aadd = mybir.AluOpType.add

            W4 = 4 * BG

            def emit_group_rest(g, pz, a, v):
                # a = sigmoid(z): cols i [0:W4], f [W4:2W4], o [2W4:3W4],
                # sg [3W4:4W4]
                nc.scalar.activation(a[:], pz[:], AF.Sigmoid)
                nc.vector.tensor_mul(tm1[g][:], a[:, W4:2 * W4], cc[g][:])
                nc.vector.scalar_tensor_tensor(
                    tm2[g][:], a[:, 3 * W4:4 * W4], -0.5, a[:, 0:W4],
                    Aadd, Amul)
                nc.vector.scalar_tensor_tensor(
                    cc[g][:], tm2[g][:], 4.0, tm1[g][:], Amul, Aadd)
                nc.scalar.activation(v[:], cc[g][:], AF.Sigmoid)

            def emit_group_h(g, a, v):
                nc.vector.scalar_tensor_tensor(
                    h16[g][:], v[:], -0.5, a[:, 2 * W4:3 * W4], Aadd, Amul)

            def body():
                for g in GL:
                    nc.gpsimd.memset(h16[g][:], 0.0)
                    nc.gpsimd.memset(cc[g][:], 0.0)
                for it in emit_prep_items(0):
                    it()
                pend_hB = None
                for c in range(nch):
                    prep_iter = iter(emit_prep_items(c + 1)) if c + 1 < nch \
                        else iter(())
                    for t in range(Tc):
                        if pend_hB is not None:
                            emit_group_h("B", *pend_hB)
                            pend_hB = None
                        pzA = emit_group_pe("A", t, c)
                        it = next(prep_iter, None)
                        if it is not None:
                            it()
                        aA = apool.tile([P, 4 * W4], f32, tag="aA",
                                        name=f"aA_{c}_{t}")
                        vA = apool.tile([P, W4], f32, tag="vA",
                                        name=f"vA_{c}_{t}")
                        emit_group_rest("A", pzA, aA, vA)
                        emit_group_h("A", aA, vA)
                        if groups == 2:
                            pzB = emit_group_pe("B", t, c)
                            aB = apool.tile([P, 4 * W4], f32, tag="aB",
                                            name=f"aB_{c}_{t}")
                            vB = apool.tile([P, W4], f32, tag="vB",
                                            name=f"vB_{c}_{t}")
                            emit_group_rest("B", pzB, aB, vB)
                            pend_hB = (aB, vB)
                        if dbg_step is not None and (c, t) == dbg_step:
                            emit_group_h("B", aB, vB)
                            pend_hB = None
                            nc.sync.dma_start(dbg_a.ap()[:, 0:32], aA[:])
                            nc.sync.dma_start(dbg_a.ap()[:, 32:64], aB[:])
                            nc.sync.dma_start(dbg_c.ap()[:, 0:8], cc["A"][:])
                            nc.sync.dma_start(dbg_c.ap()[:, 8:16], cc["B"][:])
                            nc.sync.dma_start(dbg_h.ap()[:, 0:8], h16["A"][:])
                            nc.sync.dma_start(dbg_h.ap()[:, 8:16], h16["B"][:])
                if pend_hB is not None:
                    emit_group_h("B", *pend_hB)

            if loop_reps > 0:
                with tc.For_i(0, loop_reps):
                    body()
            else:
                for _ in range(reps):
                    body()

            if groups == 2:
                nc.sync.dma_start(ho_t.ap()[:, 0:8], h16["A"][:])
                nc.sync.dma_start(ho_t.ap()[:, 8:16], h16["B"][:])
                nc.sync.dma_start(co_t.ap()[:, 0:8], cc["A"][:])
                nc.sync.dma_start(co_t.ap()[:, 8:16], cc["B"][:])
            else:
                nc.sync.dma_start(ho_t.ap(), h16["A"][:])
                nc.sync.dma_start(co_t.ap(), cc["A"][:])

    return nc




def build_program_v4(nc, T_steps=T, Tc=128, reps=1, loop_reps=0,
                     dbg_step=None):
    """v4: lockstep (all 4 batches) with hs-pair-split elementwise so h
    releases in halves and the next step's PE matmuls overlap the
    elementwise tail. Same algebra as v3 (all-sigmoid, cc=2c, fused stt,
    identity-matmul xz+bias injection).

    Layouts: z psum [128, 64] col = s*16 + hs*4 + b (s in i,f,o,sg);
    h16 [128, 16] col = k*4 + b (h/2 in fp16); cc [128, 16] col = hs*4+b.
    PE order per step: id-MM, then k0..k3 with hs01 m-tiles first within
    each k; sigma-z/sigma-c/DVE ops split per hs-pair.
    """
    assert T_steps % Tc == 0
    nch = T_steps // Tc
    NJ = Tc * BL // P

    emb_t = nc.declare_dram_parameter("emb", [V, E], f32, isOutput=False)
    W_t = nc.declare_dram_parameter("W", [E, G4], f32, isOutput=False)
    U_t = nc.declare_dram_parameter("U", [H, G4], f32, isOutput=False)
    b_t = nc.declare_dram_parameter("bvec", [G4], f32, isOutput=False)
    tok_t = nc.declare_dram_parameter("tok", [P, T_steps * BL // P], i32,
                                      isOutput=False)
    ho_t = nc.declare_dram_parameter("ho", [P, 16], f16, isOutput=True)
    co_t = nc.declare_dram_parameter("co", [P, 16], f32, isOutput=True)

    with tile.TileContext(nc) as tc:
        with (
            tc.tile_pool(name="const", bufs=1) as cpool,
            tc.tile_pool(name="ustage", bufs=2) as upool,
            tc.tile_pool(name="xrows", bufs=4) as xpool,
            tc.tile_pool(name="xtp", bufs=2) as xtpool,
            tc.tile_pool(name="av", bufs=3) as apool,
            tc.tile_pool(name="ptr", bufs=2, space="PSUM") as ptr_pool,
            tc.tile_pool(name="pxz", bufs=2, space="PSUM") as pxz_pool,
            tc.tile_pool(name="pz", bufs=2, space="PSUM") as pz_pool,
        ):
            U16 = cpool.tile([P, KH * G4], f16, tag="U16")
            W_sb = cpool.tile([P, 3 * G4], f16, tag="Wsb")
            b_sb = cpool.tile([P, KM], f32, tag="bsb")
            tok_sb = cpool.tile([P, T_steps * BL // P], i32, tag="tok")
            ident = cpool.tile([P, P], f32, tag="ident")
            ident16 = cpool.tile([P, P], f16, tag="ident16")
            h16p = [cpool.tile([P, 8], f16, tag=f"h16p{pr}",
                                 name=f"h16p{pr}") for pr in range(2)]
            cc = cpool.tile([P, 16], f32, tag="cc")
            tm1 = cpool.tile([P, 16], f32, tag="tm1")
            tm2 = cpool.tile([P, 16], f32, tag="tm2")
            xz_sb = [cpool.tile([P, Tc * 64], f16, tag=f"xz{par}",
                                name=f"xz{par}") for par in range(2)]

            make_identity(nc, ident[:])
            nc.vector.tensor_copy(ident16[:], ident[:])

            for k in range(KH):
                ust = upool.tile([P, G4], f32, tag="ustage")
                nc.sync.dma_start(ust[:], U_t.ap()[k * P:(k + 1) * P, :])
                nc.vector.tensor_copy(U16[:, k * G4:(k + 1) * G4], ust[:])
            ofs = 0
            for kk, kw in enumerate(KE_SIZES):
                wst = upool.tile([P, G4], f32, tag="ustage", name=f"wst{kk}")
                nc.sync.dma_start(wst[:kw, :], W_t.ap()[ofs:ofs + kw, :])
                nc.vector.tensor_copy(W_sb[:kw, kk * G4:(kk + 1) * G4],
                                      wst[:kw, :])
                ofs += kw
            nc.sync.dma_start(b_sb[:],
                              b_t.ap().rearrange("(m p) -> p m", p=P))
            nc.sync.dma_start(tok_sb[:], tok_t.ap())

            def emit_prep_items(c):
                xz_dst = xz_sb[c % 2]
                xT = xtpool.tile([P, 3 * Tc * BL], f16, tag="xT")

                xrs = {}
                for j in range(NJ):
                    def g(j=j):
                        xr = xpool.tile([P, E], f32, tag="xrows")
                        nc.gpsimd.indirect_dma_start(
                            out=xr[:], out_offset=None, in_=emb_t.ap(),
                            in_offset=IndirectOffsetOnAxis(
                                ap=tok_sb[:, c * NJ + j:c * NJ + j + 1],
                                axis=0),
                        )
                        xrs[j] = xr
                    yield g
                for j in range(NJ):
                    for kk, kw in enumerate(KE_SIZES):
                        def tr(j=j, kk=kk, kw=kw):
                            pt = ptr_pool.tile([P, P], f32, tag="ptr")
                            nc.tensor.transpose(
                                out=pt[:kw, :],
                                in_=xrs[j][:, kk * P:kk * P + kw],
                                identity=ident[:])
                            nc.vector.tensor_copy(
                                xT[:kw, kk * Tc * BL + j * P:
                                   kk * Tc * BL + (j + 1) * P], pt[:kw, :])
                        yield tr
                N = Tc * BL
                for m in range(KM):
                    def mm_m(m=m):
                        pxz = pxz_pool.tile([P, N], f32, tag="pxz")
                        for kk, kw in enumerate(KE_SIZES):
                            nc.tensor.matmul(
                                pxz[:],
                                W_sb[:kw, kk * G4 + m * P:
                                     kk * G4 + (m + 1) * P],
                                xT[:kw, kk * N:(kk + 1) * N],
                                start=(kk == 0), stop=(kk == 2),
                            )
                        hs = m % 4
                        slot = (hs // 2) * 32 + PERM[m // 4] * 8 + \
                            (hs % 2) * 4
                        dst = xz_dst[:].rearrange(
                            "p (t q) -> p t q", q=64)[:, :, slot:slot + 4]
                        src = pxz[:].rearrange("p (t b) -> p t b", b=BL)
                        nc.vector.tensor_scalar_add(dst, src, b_sb[:, m:m + 1])
                    yield mm_m

            # within each k: hs 0,1 m-tiles first so pair-0's z
            # completes (and its EW chain starts) before the step's end
            M_ORDER = [gk * 4 + hs for hs in (0, 1, 2, 3) for gk in range(4)]

            Amul = mybir.AluOpType.mult
            Aadd = mybir.AluOpType.add

            def emit_step(t, c):
                pzp = [pz_pool.tile([P, 32], f32, tag=f"pz{pr}",
                                    name=f"pz{pr}_{c}_{t}")
                       for pr in range(2)]
                for pr in range(2):
                    nc.tensor.matmul(
                        pzp[pr][:], ident16[:],
                        xz_sb[c % 2][:, t * 64 + pr * 32:
                                     t * 64 + pr * 32 + 32],
                        start=True, stop=False, skip_group_check=True,
                    )
                for k in range(KH):
                    for m in M_ORDER:
                        hs = m % 4
                        slot = PERM[m // 4] * 8 + (hs % 2) * 4
                        nc.tensor.matmul(
                            pzp[hs // 2][:, slot:slot + 4],
                            U16[:, k * G4 + m * P:k * G4 + (m + 1) * P],
                            h16p[k // 2][:, (k % 2) * 4:(k % 2) * 4 + 4],
                            start=False, stop=(k == KH - 1),
                            skip_group_check=True,
                        )
                v = apool.tile([P, 16], f32, tag="v", name=f"v_{c}_{t}")
                for pr in range(2):
                    # a cols: i [0:8], f [8:16], o [16:24], sg [24:32]
                    # (within-pair col = s*8 + hp*4 + b)
                    a = apool.tile([P, 32], f32, tag=f"a{pr}",
                                   name=f"a{pr}_{c}_{t}")
                    nc.scalar.activation(a[:], pzp[pr][:], AF.Sigmoid)
                    cs = slice(pr * 8, pr * 8 + 8)
                    nc.vector.tensor_mul(tm1[:, cs], a[:, 8:16], cc[:, cs])
                    nc.vector.scalar_tensor_tensor(
                        tm2[:, cs], a[:, 24:32], -0.5, a[:, 0:8],
                        Aadd, Amul)
                    nc.vector.scalar_tensor_tensor(
                        cc[:, cs], tm2[:, cs], 4.0, tm1[:, cs], Amul, Aadd)
                    nc.scalar.activation(v[:, cs], cc[:, cs], AF.Sigmoid)
                    nc.vector.scalar_tensor_tensor(
                        h16p[pr][:], v[:, cs], -0.5, a[:, 16:24],
                        Aadd, Amul)

            def body():
                nc.gpsimd.memset(h16p[0][:], 0.0)
                nc.gpsimd.memset(h16p[1][:], 0.0)
                nc.gpsimd.memset(cc[:], 0.0)
                for it in emit_prep_items(0):
                    it()
                for c in range(nch):
                    prep_iter = iter(emit_prep_items(c + 1)) if c + 1 < nch \
                        else iter(())
                    for t in range(Tc):
                        emit_step(t, c)
                        it = next(prep_iter, None)
                        if it is not None:
                            it()

            if loop_reps > 0:
                with tc.For_i(0, loop_reps):
                    body()
            else:
                for _ in range(reps):
                    body()

            nc.sync.dma_start(ho_t.ap()[:, 0:8], h16p[0][:])
            nc.sync.dma_start(ho_t.ap()[:, 8:16], h16p[1][:])
            nc.sync.dma_start(co_t.ap(), cc[:])

    return nc

_CACHE = {}

KERNEL_VER = "v4"


def _get_compiled(T_steps=T, Tc=128, ver=KERNEL_VER):
    key = (T_steps, Tc, ver)
    if key not in _CACHE:
        nc = bacc.Bacc(None, target_bir_lowering=False)
        if ver == "v4":
            build_program_v4(nc, T_steps, Tc)
        elif ver == "v3":
            build_program_v3(nc, T_steps, Tc)
        else:
            build_program(nc, T_steps, Tc)
        nc.compile()
        _CACHE[key] = nc
    return _CACHE[key]


def _scale_inputs_v3(W, U, b):
    """g-gate columns (Keras order i,f,g,o -> cols 2H:3H) doubled so one
    sigmoid yields sg = (tanh(z_g)+1)/2; U doubled overall because h is
    stored as h/2."""
    W2 = W.copy()
    W2[:, 2 * H:3 * H] *= 2.0
    b2 = b.copy()
    b2[2 * H:3 * H] *= 2.0
    U2 = 2.0 * U
    U2[:, 2 * H:3 * H] *= 2.0
    return W2, U2, b2


def unpack_v4(arr, scale):
    """[128, 16] col = hs*4 + b -> [BL, H] * scale."""
    a = np.asarray(arr).astype(np.float32).reshape(P, KH, BL)
    out = np.zeros((BL, H), np.float32)
    for bb in range(BL):
        out[bb] = a[:, :, bb].T.reshape(H)
    return out * scale


def unpack_v3(arr, scale):
    """[128, 16] (A b0,b1 | B b2,b3; col within group = hs*2+b2) ->
    [BL, H] * scale."""
    a = np.asarray(arr).astype(np.float32).reshape(P, 2, KH, 2)
    out = np.zeros((BL, H), np.float32)
    for g in range(2):
        for b2 in range(2):
            out[g * 2 + b2] = a[:, g, :, b2].T.reshape(H)
    return out * scale


def make_tok_idx(tokens_slice, T_steps=T):
    """tokens_slice [BL, T] -> [128, T*BL/128] int32, [p, j] = t-major flat[j*128+p]."""
    flat = tokens_slice.T.reshape(-1)  # index n = t*BL + b
    return np.ascontiguousarray(
        flat.reshape(T_steps * BL // P, P).T.astype(np.int32)
    )


def unpack_state(arr):
    """[128, 16] packed (p, hs*4+b) -> [BL, H]."""
    a = np.asarray(arr).astype(np.float32).reshape(P, KH, BL)
    return a.transpose(2, 1, 0).reshape(BL, H)


def kernel(tokens, emb, W, U, b):
    tokens = np.ascontiguousarray(np.asarray(tokens), dtype=np.int32)
    emb = np.ascontiguousarray(np.asarray(emb), dtype=np.float32)
    W = np.ascontiguousarray(np.asarray(W), dtype=np.float32)
    U = np.ascontiguousarray(np.asarray(U), dtype=np.float32)
    b = np.ascontiguousarray(np.asarray(b), dtype=np.float32)

    nc = _get_compiled()
    if KERNEL_VER in ("v3", "v4"):
        W, U, b = _scale_inputs_v3(W, U, b)
    in_maps = []
    for i in range(NCORES):
        in_maps.append(
            {
                "emb": emb,
                "W": W,
                "U": U,
                "bvec": b,
                "tok": make_tok_idx(tokens[i * BL:(i + 1) * BL]),
            }
        )
    res = run_bass_kernel_spmd(nc, in_maps, core_ids=list(range(NCORES))).results

    h = np.zeros((B, H), np.float32)
    c = np.zeros((B, H), np.float32)
    for i in range(NCORES):
        if KERNEL_VER == "v4":
            h[i * BL:(i + 1) * BL] = unpack_v4(res[i]["ho"], 2.0)
            c[i * BL:(i + 1) * BL] = unpack_v4(res[i]["co"], 0.5)
        elif KERNEL_VER == "v3":
            h[i * BL:(i + 1) * BL] = unpack_v3(res[i]["ho"], 2.0)
            c[i * BL:(i + 1) * BL] = unpack_v3(res[i]["co"], 0.5)
        else:
            h[i * BL:(i + 1) * BL] = unpack_state(res[i]["ho"])
            c[i * BL:(i + 1) * BL] = unpack_state(res[i]["co"])
    return h, c


def _build_run_fn(nc, n_cores=NCORES):
    """jit'd fn running the kernel once on n_cores (device-resident args)."""
    import jax
    from jax.sharding import Mesh, PartitionSpec
    from jax.experimental.shard_map import shard_map
    import concourse.mybir as mybir_
    from concourse import bass2jax

    bass2jax.install_neuronx_cc_hook()

    partition_name = nc.partition_id_tensor.name if nc.partition_id_tensor else None
    in_names, out_names, out_avals = [], [], []
    for alloc in nc.m.functions[0].allocations:
        if not isinstance(alloc, mybir_.MemoryLocationSet):
            continue
        name = alloc.memorylocations[0].name
        if alloc.kind == "ExternalInput":
            if name != partition_name:
                in_names.append(name)
        elif alloc.kind == "ExternalOutput":
            out_names.append(name)
            out_avals.append(
                jax.core.ShapedArray(
                    tuple(alloc.tensor_shape), mybir_.dt.np(alloc.dtype)
                )
            )
    n_params = len(in_names)
    all_in_names = list(in_names) + list(out_names)
    if partition_name is not None:
        all_in_names.append(partition_name)

    def _body(*args):
        operands = list(args)
        if partition_name is not None:
            operands.append(bass2jax.partition_id_tensor())
        return tuple(
            bass2jax._bass_exec_p.bind(
                *operands,
                out_avals=tuple(out_avals),
                in_names=tuple(all_in_names),
                out_names=tuple(out_names),
                lowering_input_output_aliases=(),
                sim_require_finite=True,
                sim_require_nnan=True,
                nc=nc,
            )
        )

    devices = jax.devices()[:n_cores]
    mesh = Mesh(np.asarray(devices), ("core",))
    nio = n_params + len(out_names)
    fn = jax.jit(
        shard_map(
            _body,
            mesh=mesh,
            in_specs=(PartitionSpec("core"),) * nio,
            out_specs=(PartitionSpec("core"),) * len(out_names),
            check_rep=False,
        )
    )
    return fn, in_names, out_names, out_avals


def _min_wall(nc, in_maps, calls=12):
    """Min wall-clock of a device-resident execution of nc across `calls`."""
    import time as _time
    import jax

    fn, in_names, out_names, out_avals = _build_run_fn(nc)
    concat_in = [
        np.concatenate([in_maps[c][k] for c in range(NCORES)], axis=0)
        for k in in_names
    ]
    concat_zeros = [
        np.zeros((NCORES * a.shape[0], *a.shape[1:]), a.dtype) for a in out_avals
    ]
    args = [jax.device_put(x) for x in concat_in + concat_zeros]
    o = fn(*args)
    jax.block_until_ready(o)  # compile + first exec (instruction stream cold)
    walls = []
    for _ in range(calls):
        t0 = _time.perf_counter()
        o = fn(*args)
        jax.block_until_ready(o)
        walls.append(_time.perf_counter() - t0)
    walls.sort()
    return walls[0], walls[len(walls) // 2]


def _make_in_maps(np_inputs, ver=KERNEL_VER):
    tokens = np.ascontiguousarray(np.asarray(np_inputs["tokens"]), dtype=np.int32)
    W = np.asarray(np_inputs["W"], np.float32)
    U = np.asarray(np_inputs["U"], np.float32)
    b = np.asarray(np_inputs["b"], np.float32)
    if ver in ("v3", "v4"):
        W, U, b = _scale_inputs_v3(W, U, b)
    in_maps = []
    for i in range(NCORES):
        in_maps.append(
            {
                "emb": np.asarray(np_inputs["emb"], np.float32),
                "W": W,
                "U": U,
                "bvec": b,
                "tok": make_tok_idx(tokens[i * BL:(i + 1) * BL]),
            }
        )
    return in_maps


def time_kernel_hw(np_inputs, calls=12, r_lo=2, r_hi=202, ver=KERNEL_VER,
                   n_cores=1):
    """Estimate one-pass HW time (ns) via hardware-loop amplification:
    builds the kernel with the full pass wrapped in For_i x r_lo and
    x r_hi, interleaves timed device-resident executions of both, and
    takes the median paired difference / (r_hi - r_lo). The ~40 ms
    per-call dispatch noise cancels in the pairing; the amplified signal
    (r_hi - r_lo passes) dominates the jitter.
    """
    import jax

    in_maps = _make_in_maps(np_inputs, ver)
    builder = {"v4": build_program_v4, "v3": build_program_v3}.get(
        ver, build_program)

    variants = {}
    for r in (r_lo, r_hi):
        nc = bacc.Bacc(None, target_bir_lowering=False)
        builder(nc, T, 128, loop_reps=r)
        nc.compile()
        variants[r] = nc

    fns = {}
    for r in (r_lo, r_hi):
        fn_tuple = _build_run_fn(variants[r], n_cores=n_cores)
        fn, in_names, out_names, out_avals = fn_tuple
        concat_in = [
            np.concatenate([in_maps[c][k] for c in range(n_cores)], axis=0)
            for k in in_names
        ]
        concat_zeros = [
            np.zeros((n_cores * a.shape[0], *a.shape[1:]), a.dtype)
            for a in out_avals
        ]
        args = [jax.device_put(x) for x in concat_in + concat_zeros]
        jax.block_until_ready(fn(*args))  # compile + warm
        fns[r] = (fn, args)

    import time as _t
    ds, los, his = [], [], []
    for _ in range(calls):
        t0 = _t.perf_counter()
        jax.block_until_ready(fns[r_lo][0](*fns[r_lo][1]))
        t1 = _t.perf_counter()
        jax.block_until_ready(fns[r_hi][0](*fns[r_hi][1]))
        t2 = _t.perf_counter()
        los.append(t1 - t0)
        his.append(t2 - t1)
        ds.append((t2 - t1) - (t1 - t0))
    ds.sort()
    per_pass = ds[len(ds) // 2] / (r_hi - r_lo)
    print(
        f"timing: R{r_lo} min {min(los)*1e3:.1f} ms, R{r_hi} min "
        f"{min(his)*1e3:.1f} ms, paired-median per-pass {per_pass*1e6:.1f} us"
    )
    return max(per_pass, 0.0) * 1e9


